# revision 9
# baseline (speedup 1.0000x reference)
"""Multi-head self-attention (B=2, S=2048, D=1024, H=16, causal) on 8 NeuronCores.

Sharding: core c = 4*b + g handles batch b and heads 4g..4g+3 (batch x
head-group parallel). Per core:
  - q/k projections in transposed layout  qT/kT [dh, s]  (dh on partitions)
  - v projection in natural layout [s, dh] with a fused ones-column per head
    (gives the softmax denominator for free during the AV matmul)
  - causal attention in scoresT [j, i] orientation: PE scores -> ACT exp
    (scale=1/8, no max subtraction; scores ~ N(0,1) so exp never overflows)
    -> DVE causal mask multiply on diagonal blocks -> PE AV accumulate
  - normalization of attnT by the per-query denominator via a PE ones-
    broadcast + DVE multiply during PSUM eviction
  - partial o-projection out_c = merged_c @ Wo[:, cols_c].T
Host sums the 4 partial outputs per batch (the only cross-core reduction).

All matmuls run in float32r (full-rate fp32 on the TRN2 PE).
"""

import numpy as np

import concourse.bass as bass
from concourse import bacc
import concourse.mybir as mybir
import concourse.tile as tile
from concourse import bass_utils

F32 = mybir.dt.float32
F32R = mybir.dt.float32r
EXP = mybir.ActivationFunctionType.Exp

B, S, D = 2, 2048, 1024
H, DH = 16, 64
NCORES = 8
HPG = 4                  # heads per group (per core)
M = HPG * DH             # 256 per-core head dims
DC = D // 128            # 8 contraction chunks for projections
IC = 512                 # i (query) chunk for attention
JC = 128                 # j (key) chunk for attention
SCALE = 1.0 / np.sqrt(DH)


def _build_nc():
    nc = bacc.Bacc("TRN2", target_bir_lowering=False, debug=False)

    xT_d = nc.dram_tensor("xT", [D, S], F32R, kind="ExternalInput").ap()
    wqT_d = nc.dram_tensor("wqT", [D, M], F32R, kind="ExternalInput").ap()
    wkT_d = nc.dram_tensor("wkT", [D, M], F32R, kind="ExternalInput").ap()
    wvT_d = nc.dram_tensor("wvT", [D, M], F32R, kind="ExternalInput").ap()
    woT_d = nc.dram_tensor("woT", [M, D], F32R, kind="ExternalInput").ap()
    mask_d = nc.dram_tensor("mask", [JC, 896], F32, kind="ExternalInput").ap()
    onesa_d = nc.dram_tensor("ones_a", [1, 64], F32R, kind="ExternalInput").ap()
    onesb_d = nc.dram_tensor("ones_b", [JC, HPG], F32R, kind="ExternalInput").ap()
    out_d = nc.dram_tensor("out", [S, D], F32, kind="ExternalOutput").ap()

    with tile.TileContext(nc) as tc:
        _body(tc, xT_d, wqT_d, wkT_d, wvT_d, woT_d, mask_d, onesa_d, onesb_d, out_d)
    nc.compile()
    return nc


def _body(tc, xT_d, wqT_d, wkT_d, wvT_d, woT_d, mask_d, onesa_d, onesb_d, out_d):
    nc = tc.nc
    from contextlib import ExitStack
    ctx = ExitStack()
    with ctx:
        p_x = ctx.enter_context(tc.tile_pool(name="x", bufs=DC))
        p_w = ctx.enter_context(tc.tile_pool(name="w", bufs=DC))
        p_wo = ctx.enter_context(tc.tile_pool(name="wo", bufs=2))
        p_qk = ctx.enter_context(tc.tile_pool(name="qk", bufs=2))
        p_v = ctx.enter_context(tc.tile_pool(name="v", bufs=S // JC))
        p_mg = ctx.enter_context(tc.tile_pool(name="mg", bufs=2))
        p_probs = ctx.enter_context(tc.tile_pool(name="probs", bufs=4))
        p_small = ctx.enter_context(tc.tile_pool(name="small", bufs=2))
        p_mask = ctx.enter_context(tc.tile_pool(name="mask", bufs=1))
        p_ones = ctx.enter_context(tc.tile_pool(name="ones", bufs=1))
        p_ostg = ctx.enter_context(tc.tile_pool(name="ostg", bufs=2))

        ps_big = ctx.enter_context(tc.tile_pool(name="psb", bufs=2, space="PSUM"))
        ps_sc = ctx.enter_context(tc.tile_pool(name="pss", bufs=3, space="PSUM"))
        ps_at = ctx.enter_context(tc.tile_pool(name="psa", bufs=2, space="PSUM"))

        # ---- input loads ----
        x_t = []
        for dc in range(DC):
            t = p_x.tile([128, S], F32R, tag="x")
            nc.sync.dma_start(t[:], xT_d[dc * 128:(dc + 1) * 128, :])
            x_t.append(t)
        wq_t, wk_t, wv_t = [], [], []
        for w_d, lst, tg in ((wqT_d, wq_t, "wq"), (wkT_d, wk_t, "wk"), (wvT_d, wv_t, "wv")):
            for dc in range(DC):
                t = p_w.tile([128, M], F32R, tag=tg)
                nc.sync.dma_start(t[:], w_d[dc * 128:(dc + 1) * 128, :])
                lst.append(t)
        wo_t = []
        for kc in range(2):
            t = p_wo.tile([128, D], F32R, tag="wo")
            nc.sync.dma_start(t[:], woT_d[kc * 128:(kc + 1) * 128, :])
            wo_t.append(t)
        mask_t = p_mask.tile([JC, 896], F32, tag="mask")
        nc.sync.dma_start(mask_t[:], mask_d[:])
        ones_t = p_ones.tile([1, 64], F32R, tag="ones")
        nc.sync.dma_start(ones_t[:], onesa_d[:])
        onesb_t = p_ones.tile([JC, HPG], F32R, tag="onesb")
        nc.sync.dma_start(onesb_t[:], onesb_d[:])

        # ---- q/k projections (transposed): qT[m, s] = sum_d WqT[d, m] xT[d, s]
        q_t, k_t = [], []
        for w_list, q_list, tg in ((wq_t, q_t, "qT"), (wk_t, k_t, "kT")):
            for mc in range(M // 128):
                dst = p_qk.tile([128, S], F32R, tag=tg)
                for s4 in range(S // 512):
                    ps = ps_big.tile([128, 512], F32, tag="proj")
                    for dc in range(DC):
                        nc.tensor.matmul(
                            ps[:],
                            w_list[dc][:, mc * 128:(mc + 1) * 128],
                            x_t[dc][:, s4 * 512:(s4 + 1) * 512],
                            start=(dc == 0), stop=(dc == DC - 1))
                    nc.any.tensor_copy(dst[:, s4 * 512:(s4 + 1) * 512], ps[:])
                q_list.append(dst)

        # ---- v projection (natural): v[s, m] = sum_d xT[d, s] wvT[d, m]
        # tile layout [128 j, 260]: per head h cols h*65..h*65+63 = v, col h*65+64 = 1.0
        v_t = []
        for sc in range(S // JC):
            vt = p_v.tile([JC, HPG * (DH + 1)], F32R, tag="v")
            nc.vector.tensor_copy(
                vt[:].rearrange("p (h e) -> p h e", h=HPG)[:, :, DH:DH + 1].squeeze(2),
                onesb_t[:])
            ps = ps_big.tile([128, 512], F32, tag="proj")
            for dc in range(DC):
                nc.tensor.matmul(
                    ps[:, 0:M],
                    x_t[dc][:, sc * 128:(sc + 1) * 128],
                    wv_t[dc][:],
                    start=(dc == 0), stop=(dc == DC - 1))
            src = ps[:, 0:M].rearrange("p (h d) -> p h d", h=HPG)
            dst = vt[:].rearrange("p (h e) -> p h e", h=HPG)[:, :, 0:DH]
            nc.vector.tensor_copy(dst, src)
            v_t.append(vt)

        # ---- attention, scoresT orientation ----
        mg_t = [p_mg.tile([128, S], F32R, tag="mgT", name=f"mg{i}")
                for i in range(M // 128)]
        for h in range(HPG):
            qk_tile = h // 2
            prow = 64 * (h % 2)
            for ic in range(S // IC):
                njc = (ic * IC) // JC + IC // JC  # causal: j chunks 0..njc-1
                at_ps = ps_at.tile([DH + 1, IC], F32, tag="attn")
                for jc in range(njc):
                    sc_ps = ps_sc.tile([128, IC], F32, tag="scores")
                    nc.tensor.matmul(
                        sc_ps[:],
                        k_t[qk_tile][prow:prow + DH, jc * JC:(jc + 1) * JC],
                        q_t[qk_tile][prow:prow + DH, ic * IC:(ic + 1) * IC],
                        start=True, stop=True)
                    pr = p_probs.tile([JC, IC], F32R, tag="probs")
                    nc.scalar.activation(pr[:], sc_ps[:], EXP, scale=SCALE)
                    delta = jc * JC - ic * IC
                    if delta >= 0:  # diagonal block: mask j+delta > i
                        off = 384 - delta
                        nc.vector.tensor_mul(
                            pr[:], pr[:], mask_t[:, off:off + IC])
                    nc.tensor.matmul(
                        at_ps[:],
                        v_t[jc][:, h * (DH + 1):(h + 1) * (DH + 1)],
                        pr[:],
                        start=(jc == 0), stop=(jc == njc - 1))
                # normalize rows 0..63 by row 64 (denominator), evict to mergedT
                rc = p_small.tile([1, IC], F32R, tag="recip")
                with nc.allow_low_precision(reason="f32r rounding of softmax denom reciprocal"):
                    nc.vector.reciprocal(rc[:], at_ps[DH:DH + 1, :])
                bc_ps = ps_sc.tile([DH, IC], F32, tag="scores")
                nc.tensor.matmul(bc_ps[:], ones_t[:], rc[:],
                                 start=True, stop=True)
                bc_sb = p_small.tile([DH, IC], F32, tag="bcast")
                nc.vector.tensor_copy(bc_sb[:], bc_ps[:])
                nc.vector.tensor_mul(
                    mg_t[qk_tile][prow:prow + DH, ic * IC:(ic + 1) * IC],
                    at_ps[0:DH, :], bc_sb[:])

        # ---- partial o-projection: out[s, o] = sum_k mergedT[k, s] woT[k, o]
        for sc in range(S // 128):
            stg = p_ostg.tile([128, D], F32, tag="ostg")
            for nn in range(2):
                ps = ps_big.tile([128, 512], F32, tag="proj")
                for kc in range(2):
                    nc.tensor.matmul(
                        ps[:],
                        mg_t[kc][:, sc * 128:(sc + 1) * 128],
                        wo_t[kc][:, nn * 512:(nn + 1) * 512],
                        start=(kc == 0), stop=(kc == 1))
                nc.any.tensor_copy(stg[:, nn * 512:(nn + 1) * 512], ps[:])
            nc.sync.dma_start(out_d[sc * 128:(sc + 1) * 128, :], stg[:])


_NC_CACHE = None


def _get_nc():
    global _NC_CACHE
    if _NC_CACHE is None:
        _NC_CACHE = _build_nc()
    return _NC_CACHE


def _causal_mask_tile():
    # BIGMASK[j, c] = 1.0 if j <= c - 384 else 0.0, shape [128, 896].
    # Diagonal block at delta = j_base - i_base uses cols [384-delta, 384-delta+512).
    j = np.arange(JC)[:, None]
    c = np.arange(896)[None, :]
    return (j <= c - 384).astype(np.float32)


def _prepare_in_maps(inputs):
    x = np.asarray(inputs["in_features"], dtype=np.float32)
    wqT = np.ascontiguousarray(np.asarray(inputs["q_proj_weight"], np.float32).T)
    wkT = np.ascontiguousarray(np.asarray(inputs["k_proj_weight"], np.float32).T)
    wvT = np.ascontiguousarray(np.asarray(inputs["v_proj_weight"], np.float32).T)
    woT = np.ascontiguousarray(np.asarray(inputs["o_proj_weight"], np.float32).T)
    xT = [np.ascontiguousarray(x[b].T) for b in range(B)]
    mask = _causal_mask_tile()

    in_maps = []
    for c in range(NCORES):
        b, g = divmod(c, HPG)
        ms = slice(g * M, (g + 1) * M)
        in_maps.append({
            "xT": xT[b],
            "wqT": np.ascontiguousarray(wqT[:, ms]),
            "wkT": np.ascontiguousarray(wkT[:, ms]),
            "wvT": np.ascontiguousarray(wvT[:, ms]),
            "woT": np.ascontiguousarray(woT[ms, :]),
            "mask": mask,
            "ones_a": np.ones((1, 64), np.float32),
            "ones_b": np.ones((JC, HPG), np.float32),
        })
    return in_maps


def kernel(q_proj_weight, k_proj_weight, v_proj_weight, o_proj_weight, in_features):
    in_dtype = np.asarray(in_features).dtype
    in_maps = _prepare_in_maps({
        "q_proj_weight": q_proj_weight,
        "k_proj_weight": k_proj_weight,
        "v_proj_weight": v_proj_weight,
        "o_proj_weight": o_proj_weight,
        "in_features": in_features,
    })
    nc = _get_nc()
    res = bass_utils.run_bass_kernel_spmd(nc, in_maps, core_ids=list(range(NCORES)))
    out = np.zeros((B, S, D), dtype=np.float32)
    for c in range(NCORES):
        out[c // HPG] += res.results[c]["out"]
    return out.astype(in_dtype)


# revision 12
# speedup vs baseline: 1.1769x; 1.1769x over previous
"""Multi-head self-attention (B=2, S=2048, D=1024, H=16, causal) on 8 NeuronCores.

Sharding: core c = 4*b + g handles batch b and heads 4g..4g+3 (batch x
head-group parallel). Per core:
  - q/k projections in transposed layout  qT/kT [dh, s]  (dh on partitions)
  - v projection in natural layout [s, dh] with a fused ones-column per head
    (gives the softmax denominator for free during the AV matmul)
  - causal attention in scoresT [j, i] orientation: PE scores -> ACT exp
    (scale=1/8, no max subtraction; scores ~ N(0,1) so exp never overflows)
    -> DVE causal mask multiply on diagonal blocks -> PE AV accumulate
  - normalization of attnT by the per-query denominator via a PE ones-
    broadcast + DVE multiply during PSUM eviction
  - partial o-projection out_c = merged_c @ Wo[:, cols_c].T
Host sums the 4 partial outputs per batch (the only cross-core reduction).

All matmuls run in float32r (full-rate fp32 on the TRN2 PE).
"""

import numpy as np

import concourse.bass as bass
from concourse import bacc
import concourse.mybir as mybir
import concourse.tile as tile
from concourse import bass_utils

F32 = mybir.dt.float32
F32R = mybir.dt.float32r
EXP = mybir.ActivationFunctionType.Exp

B, S, D = 2, 2048, 1024
H, DH = 16, 64
NCORES = 8
HPG = 4                  # heads per group (per core)
M = HPG * DH             # 256 per-core head dims
DC = D // 128            # 8 contraction chunks for projections
IC = 512                 # i (query) chunk for attention
JC = 128                 # j (key) chunk for attention
SCALE = 1.0 / np.sqrt(DH)


def _build_nc():
    nc = bacc.Bacc("TRN2", target_bir_lowering=False, debug=False)

    xT_d = nc.dram_tensor("xT", [D, S], F32R, kind="ExternalInput").ap()
    wqT_d = nc.dram_tensor("wqT", [D, M], F32R, kind="ExternalInput").ap()
    wkT_d = nc.dram_tensor("wkT", [D, M], F32R, kind="ExternalInput").ap()
    wvT_d = nc.dram_tensor("wvT", [D, M], F32R, kind="ExternalInput").ap()
    woT_d = nc.dram_tensor("woT", [M, D], F32R, kind="ExternalInput").ap()
    mask_d = nc.dram_tensor("mask", [JC, 896], F32, kind="ExternalInput").ap()
    onesa_d = nc.dram_tensor("ones_a", [1, 64], F32R, kind="ExternalInput").ap()
    onesb_d = nc.dram_tensor("ones_b", [JC, HPG], F32R, kind="ExternalInput").ap()
    out_d = nc.dram_tensor("out", [S, D], F32, kind="ExternalOutput").ap()

    with tile.TileContext(nc) as tc:
        _body(tc, xT_d, wqT_d, wkT_d, wvT_d, woT_d, mask_d, onesa_d, onesb_d, out_d)
    nc.compile()
    return nc


def _body(tc, xT_d, wqT_d, wkT_d, wvT_d, woT_d, mask_d, onesa_d, onesb_d, out_d):
    nc = tc.nc
    from contextlib import ExitStack
    ctx = ExitStack()
    with ctx:
        p_x = ctx.enter_context(tc.tile_pool(name="x", bufs=DC))
        p_w = ctx.enter_context(tc.tile_pool(name="w", bufs=DC))
        p_wo = ctx.enter_context(tc.tile_pool(name="wo", bufs=2))
        p_qk = ctx.enter_context(tc.tile_pool(name="qk", bufs=2))
        p_v = ctx.enter_context(tc.tile_pool(name="v", bufs=S // JC))
        p_mg = ctx.enter_context(tc.tile_pool(name="mg", bufs=2))
        p_probs = ctx.enter_context(tc.tile_pool(name="probs", bufs=6))
        p_small = ctx.enter_context(tc.tile_pool(name="small", bufs=2))
        p_mask = ctx.enter_context(tc.tile_pool(name="mask", bufs=1))
        p_ones = ctx.enter_context(tc.tile_pool(name="ones", bufs=1))
        p_ostg = ctx.enter_context(tc.tile_pool(name="ostg", bufs=2))

        ps_big = ctx.enter_context(tc.tile_pool(name="psb", bufs=2, space="PSUM"))
        ps_sc = ctx.enter_context(tc.tile_pool(name="pss", bufs=3, space="PSUM"))
        ps_at = ctx.enter_context(tc.tile_pool(name="psa", bufs=3, space="PSUM"))

        # ---- input loads ----
        x_t = []
        for dc in range(DC):
            t = p_x.tile([128, S], F32R, tag="x")
            nc.sync.dma_start(t[:], xT_d[dc * 128:(dc + 1) * 128, :])
            x_t.append(t)
        wq_t, wk_t, wv_t = [], [], []
        for w_d, lst, tg in ((wqT_d, wq_t, "wq"), (wkT_d, wk_t, "wk"), (wvT_d, wv_t, "wv")):
            for dc in range(DC):
                t = p_w.tile([128, M], F32R, tag=tg)
                nc.sync.dma_start(t[:], w_d[dc * 128:(dc + 1) * 128, :])
                lst.append(t)
        wo_t = []
        for kc in range(2):
            t = p_wo.tile([128, D], F32R, tag="wo")
            nc.sync.dma_start(t[:], woT_d[kc * 128:(kc + 1) * 128, :])
            wo_t.append(t)
        mask_t = p_mask.tile([JC, 896], F32, tag="mask")
        nc.sync.dma_start(mask_t[:], mask_d[:])
        ones_t = p_ones.tile([1, 64], F32R, tag="ones")
        nc.sync.dma_start(ones_t[:], onesa_d[:])
        onesb_t = p_ones.tile([JC, HPG], F32R, tag="onesb")
        nc.sync.dma_start(onesb_t[:], onesb_d[:])

        # ---- q/k projections (transposed): qT[m, s] = sum_d WqT[d, m] xT[d, s]
        q_t, k_t = [], []
        for w_list, q_list, tg in ((wq_t, q_t, "qT"), (wk_t, k_t, "kT")):
            for mc in range(M // 128):
                dst = p_qk.tile([128, S], F32R, tag=tg)
                for s4 in range(S // 512):
                    ps = ps_big.tile([128, 512], F32, tag="proj")
                    for dc in range(DC):
                        nc.tensor.matmul(
                            ps[:],
                            w_list[dc][:, mc * 128:(mc + 1) * 128],
                            x_t[dc][:, s4 * 512:(s4 + 1) * 512],
                            start=(dc == 0), stop=(dc == DC - 1))
                    nc.any.tensor_copy(dst[:, s4 * 512:(s4 + 1) * 512], ps[:])
                q_list.append(dst)

        # ---- v projection (natural): v[s, m] = sum_d xT[d, s] wvT[d, m]
        # tile layout [128 j, 260]: per head h cols h*65..h*65+63 = v, col h*65+64 = 1.0
        v_t = []
        for sc in range(S // JC):
            vt = p_v.tile([JC, HPG * (DH + 1)], F32R, tag="v")
            nc.vector.tensor_copy(
                vt[:].rearrange("p (h e) -> p h e", h=HPG)[:, :, DH:DH + 1].squeeze(2),
                onesb_t[:])
            ps = ps_big.tile([128, 512], F32, tag="proj")
            for dc in range(DC):
                nc.tensor.matmul(
                    ps[:, 0:M],
                    x_t[dc][:, sc * 128:(sc + 1) * 128],
                    wv_t[dc][:],
                    start=(dc == 0), stop=(dc == DC - 1))
            src = ps[:, 0:M].rearrange("p (h d) -> p h d", h=HPG)
            dst = vt[:].rearrange("p (h e) -> p h e", h=HPG)[:, :, 0:DH]
            nc.vector.tensor_copy(dst, src)
            v_t.append(vt)

        # ---- attention, scoresT orientation ----
        # Emission is software-pipelined: the normalize/evict of a group
        # (reciprocal -> PE ones-broadcast -> DVE mul) is emitted one group
        # later so the reciprocal never stalls the in-order PE stream.
        # o-projection blocks are emitted as soon as their i-range has all
        # 4 heads normalized.
        mg_t = [p_mg.tile([128, S], F32R, tag="mgT", name=f"mg{i}")
                for i in range(M // 128)]

        def attend(h, ic):
            qk_tile = h // 2
            prow = 64 * (h % 2)
            njc = (ic * IC) // JC + IC // JC  # causal: j chunks 0..njc-1
            at_ps = ps_at.tile([DH + 1, IC], F32, tag="attn")
            for jc in range(njc):
                sc_ps = ps_sc.tile([128, IC], F32, tag="scores")
                nc.tensor.matmul(
                    sc_ps[:],
                    k_t[qk_tile][prow:prow + DH, jc * JC:(jc + 1) * JC],
                    q_t[qk_tile][prow:prow + DH, ic * IC:(ic + 1) * IC],
                    start=True, stop=True)
                pr = p_probs.tile([JC, IC], F32R, tag="probs")
                nc.scalar.activation(pr[:], sc_ps[:], EXP, scale=SCALE)
                delta = jc * JC - ic * IC
                if delta >= 0:  # diagonal block: mask j+delta > i
                    off = 384 - delta
                    nc.vector.tensor_mul(
                        pr[:], pr[:], mask_t[:, off:off + IC])
                nc.tensor.matmul(
                    at_ps[:],
                    v_t[jc][:, h * (DH + 1):(h + 1) * (DH + 1)],
                    pr[:],
                    start=(jc == 0), stop=(jc == njc - 1))
            return at_ps

        def normalize(h, ic, at_ps):
            # rows 0..63 / row 64 (denominator), evicted into mergedT
            qk_tile = h // 2
            prow = 64 * (h % 2)
            rc = p_small.tile([1, IC], F32R, tag="recip")
            with nc.allow_low_precision(reason="f32r rounding of softmax denom reciprocal"):
                nc.vector.reciprocal(rc[:], at_ps[DH:DH + 1, :])
            bc_ps = ps_sc.tile([DH, IC], F32, tag="scores")
            nc.tensor.matmul(bc_ps[:], ones_t[:], rc[:], start=True, stop=True)
            bc_sb = p_small.tile([DH, IC], F32, tag="bcast")
            nc.scalar.copy(bc_sb[:], bc_ps[:])
            nc.vector.tensor_mul(
                mg_t[qk_tile][prow:prow + DH, ic * IC:(ic + 1) * IC],
                at_ps[0:DH, :], bc_sb[:])

        def oproj(sc):
            # out[s, o] = sum_k mergedT[k, s] woT[k, o] for s-chunk sc
            stg = p_ostg.tile([128, D], F32, tag="ostg")
            for nn in range(2):
                ps = ps_big.tile([128, 512], F32, tag="proj")
                for kc in range(2):
                    nc.tensor.matmul(
                        ps[:],
                        mg_t[kc][:, sc * 128:(sc + 1) * 128],
                        wo_t[kc][:, nn * 512:(nn + 1) * 512],
                        start=(kc == 0), stop=(kc == 1))
                nc.any.tensor_copy(stg[:, nn * 512:(nn + 1) * 512], ps[:])
            nc.sync.dma_start(out_d[sc * 128:(sc + 1) * 128, :], stg[:])

        groups = [(h, ic) for ic in range(S // IC) for h in range(HPG)]
        pending = None
        for g in groups:
            at = attend(*g)
            if pending is not None:
                normalize(*pending)
                if pending[0] == HPG - 1:  # last head of its ic: mergedT
                    for sc in range(4 * pending[1], 4 * pending[1] + 4):
                        oproj(sc)          # cols for this ic fully written
            pending = (g[0], g[1], at)
        normalize(*pending)
        for sc in range(4 * pending[1], 4 * pending[1] + 4):
            oproj(sc)


_NC_CACHE = None


def _get_nc():
    global _NC_CACHE
    if _NC_CACHE is None:
        _NC_CACHE = _build_nc()
    return _NC_CACHE


def _causal_mask_tile():
    # BIGMASK[j, c] = 1.0 if j <= c - 384 else 0.0, shape [128, 896].
    # Diagonal block at delta = j_base - i_base uses cols [384-delta, 384-delta+512).
    j = np.arange(JC)[:, None]
    c = np.arange(896)[None, :]
    return (j <= c - 384).astype(np.float32)


def _prepare_in_maps(inputs):
    x = np.asarray(inputs["in_features"], dtype=np.float32)
    wqT = np.ascontiguousarray(np.asarray(inputs["q_proj_weight"], np.float32).T)
    wkT = np.ascontiguousarray(np.asarray(inputs["k_proj_weight"], np.float32).T)
    wvT = np.ascontiguousarray(np.asarray(inputs["v_proj_weight"], np.float32).T)
    woT = np.ascontiguousarray(np.asarray(inputs["o_proj_weight"], np.float32).T)
    xT = [np.ascontiguousarray(x[b].T) for b in range(B)]
    mask = _causal_mask_tile()

    in_maps = []
    for c in range(NCORES):
        b, g = divmod(c, HPG)
        ms = slice(g * M, (g + 1) * M)
        in_maps.append({
            "xT": xT[b],
            "wqT": np.ascontiguousarray(wqT[:, ms]),
            "wkT": np.ascontiguousarray(wkT[:, ms]),
            "wvT": np.ascontiguousarray(wvT[:, ms]),
            "woT": np.ascontiguousarray(woT[ms, :]),
            "mask": mask,
            "ones_a": np.ones((1, 64), np.float32),
            "ones_b": np.ones((JC, HPG), np.float32),
        })
    return in_maps


def kernel(q_proj_weight, k_proj_weight, v_proj_weight, o_proj_weight, in_features):
    in_dtype = np.asarray(in_features).dtype
    in_maps = _prepare_in_maps({
        "q_proj_weight": q_proj_weight,
        "k_proj_weight": k_proj_weight,
        "v_proj_weight": v_proj_weight,
        "o_proj_weight": o_proj_weight,
        "in_features": in_features,
    })
    nc = _get_nc()
    res = bass_utils.run_bass_kernel_spmd(nc, in_maps, core_ids=list(range(NCORES)))
    out = np.zeros((B, S, D), dtype=np.float32)
    for c in range(NCORES):
        out[c // HPG] += res.results[c]["out"]
    return out.astype(in_dtype)


# revision 13
# speedup vs baseline: 1.2146x; 1.0321x over previous
"""Multi-head self-attention (B=2, S=2048, D=1024, H=16, causal) on 8 NeuronCores.

Sharding: core c = 4*b + g handles batch b and heads 4g..4g+3 (batch x
head-group parallel). Per core:
  - q/k projections in transposed layout  qT/kT [dh, s]  (dh on partitions)
  - v projection in natural layout [s, dh] with a fused ones-column per head
    (gives the softmax denominator for free during the AV matmul)
  - causal attention in scoresT [j, i] orientation: PE scores -> ACT exp
    (scale=1/8, no max subtraction; scores ~ N(0,1) so exp never overflows)
    -> DVE causal mask multiply on diagonal blocks -> PE AV accumulate
  - normalization of attnT by the per-query denominator via a PE ones-
    broadcast + DVE multiply during PSUM eviction
  - partial o-projection out_c = merged_c @ Wo[:, cols_c].T
Host sums the 4 partial outputs per batch (the only cross-core reduction).

All matmuls run in float32r (full-rate fp32 on the TRN2 PE).
"""

import numpy as np

import concourse.bass as bass
from concourse import bacc
import concourse.mybir as mybir
import concourse.tile as tile
from concourse import bass_utils

F32 = mybir.dt.float32
F32R = mybir.dt.float32r
EXP = mybir.ActivationFunctionType.Exp

B, S, D = 2, 2048, 1024
H, DH = 16, 64
NCORES = 8
HPG = 4                  # heads per group (per core)
M = HPG * DH             # 256 per-core head dims
DC = D // 128            # 8 contraction chunks for projections
IC = 512                 # i (query) chunk for attention
JC = 128                 # j (key) chunk for attention
SCALE = 1.0 / np.sqrt(DH)


def _build_nc():
    nc = bacc.Bacc("TRN2", target_bir_lowering=False, debug=False)

    xT_d = nc.dram_tensor("xT", [D, S], F32R, kind="ExternalInput").ap()
    wqT_d = nc.dram_tensor("wqT", [D, M], F32R, kind="ExternalInput").ap()
    wkT_d = nc.dram_tensor("wkT", [D, M], F32R, kind="ExternalInput").ap()
    wvT_d = nc.dram_tensor("wvT", [D, M], F32R, kind="ExternalInput").ap()
    woT_d = nc.dram_tensor("woT", [M, D], F32R, kind="ExternalInput").ap()
    mask_d = nc.dram_tensor("mask", [JC, 896], F32, kind="ExternalInput").ap()
    onesa_d = nc.dram_tensor("ones_a", [1, 64], F32R, kind="ExternalInput").ap()
    onesb_d = nc.dram_tensor("ones_b", [JC, HPG], F32R, kind="ExternalInput").ap()
    out_d = nc.dram_tensor("out", [S, D], F32, kind="ExternalOutput").ap()

    with tile.TileContext(nc) as tc:
        _body(tc, xT_d, wqT_d, wkT_d, wvT_d, woT_d, mask_d, onesa_d, onesb_d, out_d)
    nc.compile()
    return nc


def _body(tc, xT_d, wqT_d, wkT_d, wvT_d, woT_d, mask_d, onesa_d, onesb_d, out_d):
    nc = tc.nc
    from contextlib import ExitStack
    ctx = ExitStack()
    with ctx:
        p_x = ctx.enter_context(tc.tile_pool(name="x", bufs=DC))
        p_w = ctx.enter_context(tc.tile_pool(name="w", bufs=DC))
        p_wo = ctx.enter_context(tc.tile_pool(name="wo", bufs=2))
        p_qk = ctx.enter_context(tc.tile_pool(name="qk", bufs=2))
        p_v = ctx.enter_context(tc.tile_pool(name="v", bufs=S // JC))
        p_mg = ctx.enter_context(tc.tile_pool(name="mg", bufs=2))
        p_probs = ctx.enter_context(tc.tile_pool(name="probs", bufs=6))
        p_small = ctx.enter_context(tc.tile_pool(name="small", bufs=2))
        p_mask = ctx.enter_context(tc.tile_pool(name="mask", bufs=1))
        p_ones = ctx.enter_context(tc.tile_pool(name="ones", bufs=1))
        p_ostg = ctx.enter_context(tc.tile_pool(name="ostg", bufs=2))

        ps_big = ctx.enter_context(tc.tile_pool(name="psb", bufs=2, space="PSUM"))
        ps_sc = ctx.enter_context(tc.tile_pool(name="pss", bufs=3, space="PSUM"))
        ps_at = ctx.enter_context(tc.tile_pool(name="psa", bufs=3, space="PSUM"))

        # ---- input loads ----
        x_t = []
        for dc in range(DC):
            t = p_x.tile([128, S], F32R, tag="x")
            nc.sync.dma_start(t[:], xT_d[dc * 128:(dc + 1) * 128, :])
            x_t.append(t)
        wq_t, wk_t, wv_t = [], [], []
        for w_d, lst, tg in ((wqT_d, wq_t, "wq"), (wkT_d, wk_t, "wk"), (wvT_d, wv_t, "wv")):
            for dc in range(DC):
                t = p_w.tile([128, M], F32R, tag=tg)
                nc.sync.dma_start(t[:], w_d[dc * 128:(dc + 1) * 128, :])
                lst.append(t)
        wo_t = []
        for kc in range(2):
            t = p_wo.tile([128, D], F32R, tag="wo")
            nc.sync.dma_start(t[:], woT_d[kc * 128:(kc + 1) * 128, :])
            wo_t.append(t)
        mask_t = p_mask.tile([JC, 896], F32, tag="mask")
        nc.sync.dma_start(mask_t[:], mask_d[:])
        ones_t = p_ones.tile([1, 64], F32R, tag="ones")
        nc.sync.dma_start(ones_t[:], onesa_d[:])
        onesb_t = p_ones.tile([JC, HPG], F32R, tag="onesb")
        nc.sync.dma_start(onesb_t[:], onesb_d[:])

        # ---- q/k projections (transposed): qT[m, s] = sum_d WqT[d, m] xT[d, s]
        q_t, k_t = [], []
        for w_list, q_list, tg in ((wq_t, q_t, "qT"), (wk_t, k_t, "kT")):
            for mc in range(M // 128):
                dst = p_qk.tile([128, S], F32R, tag=tg)
                for s4 in range(S // 512):
                    ps = ps_big.tile([128, 512], F32, tag="proj")
                    for dc in range(DC):
                        nc.tensor.matmul(
                            ps[:],
                            w_list[dc][:, mc * 128:(mc + 1) * 128],
                            x_t[dc][:, s4 * 512:(s4 + 1) * 512],
                            start=(dc == 0), stop=(dc == DC - 1))
                    nc.any.tensor_copy(dst[:, s4 * 512:(s4 + 1) * 512], ps[:])
                q_list.append(dst)

        # ---- v projection (natural): v[s, m] = sum_d xT[d, s] wvT[d, m]
        # tile layout [128 j, 260]: per head h cols h*65..h*65+63 = v, col h*65+64 = 1.0
        v_t = []
        for sc in range(S // JC):
            vt = p_v.tile([JC, HPG * (DH + 1)], F32R, tag="v")
            nc.vector.tensor_copy(
                vt[:].rearrange("p (h e) -> p h e", h=HPG)[:, :, DH:DH + 1].squeeze(2),
                onesb_t[:])
            ps = ps_big.tile([128, 512], F32, tag="proj")
            for dc in range(DC):
                nc.tensor.matmul(
                    ps[:, 0:M],
                    x_t[dc][:, sc * 128:(sc + 1) * 128],
                    wv_t[dc][:],
                    start=(dc == 0), stop=(dc == DC - 1))
            src = ps[:, 0:M].rearrange("p (h d) -> p h d", h=HPG)
            dst = vt[:].rearrange("p (h e) -> p h e", h=HPG)[:, :, 0:DH]
            nc.vector.tensor_copy(dst, src)
            v_t.append(vt)

        # ---- attention, scoresT orientation ----
        # Emission is software-pipelined: the normalize/evict of a group
        # (reciprocal -> PE ones-broadcast -> DVE mul) is emitted one group
        # later so the reciprocal never stalls the in-order PE stream.
        # o-projection blocks are emitted as soon as their i-range has all
        # 4 heads normalized.
        mg_t = [p_mg.tile([128, S], F32R, tag="mgT", name=f"mg{i}")
                for i in range(M // 128)]

        def attend(h, ic):
            qk_tile = h // 2
            prow = 64 * (h % 2)
            njc = (ic * IC) // JC + IC // JC  # causal: j chunks 0..njc-1
            at_ps = ps_at.tile([DH + 1, IC], F32, tag="attn")
            for jc in range(njc):
                sc_ps = ps_sc.tile([128, IC], F32, tag="scores")
                nc.tensor.matmul(
                    sc_ps[:],
                    k_t[qk_tile][prow:prow + DH, jc * JC:(jc + 1) * JC],
                    q_t[qk_tile][prow:prow + DH, ic * IC:(ic + 1) * IC],
                    start=True, stop=True)
                pr = p_probs.tile([JC, IC], F32R, tag="probs")
                nc.scalar.activation(pr[:], sc_ps[:], EXP, scale=SCALE)
                delta = jc * JC - ic * IC
                if delta >= 0:  # diagonal block: mask j+delta > i
                    off = 384 - delta
                    nc.vector.tensor_mul(
                        pr[:], pr[:], mask_t[:, off:off + IC])
                nc.tensor.matmul(
                    at_ps[:],
                    v_t[jc][:, h * (DH + 1):(h + 1) * (DH + 1)],
                    pr[:],
                    start=(jc == 0), stop=(jc == njc - 1))
            return at_ps

        def normalize(h, ic, at_ps):
            # rows 0..63 / row 64 (denominator), evicted into mergedT
            qk_tile = h // 2
            prow = 64 * (h % 2)
            rc = p_small.tile([1, IC], F32R, tag="recip")
            with nc.allow_low_precision(reason="f32r rounding of softmax denom reciprocal"):
                nc.vector.reciprocal(rc[:], at_ps[DH:DH + 1, :])
            bc_ps = ps_sc.tile([DH, IC], F32, tag="scores")
            nc.tensor.matmul(bc_ps[:], ones_t[:], rc[:], start=True, stop=True)
            bc_sb = p_small.tile([DH, IC], F32, tag="bcast")
            nc.scalar.copy(bc_sb[:], bc_ps[:])
            nc.vector.tensor_mul(
                mg_t[qk_tile][prow:prow + DH, ic * IC:(ic + 1) * IC],
                at_ps[0:DH, :], bc_sb[:])

        def oproj(sc):
            # out[s, o] = sum_k mergedT[k, s] woT[k, o] for s-chunk sc
            stg = p_ostg.tile([128, D], F32, tag="ostg")
            for nn in range(2):
                ps = ps_big.tile([128, 512], F32, tag="proj")
                for kc in range(2):
                    nc.tensor.matmul(
                        ps[:],
                        mg_t[kc][:, sc * 128:(sc + 1) * 128],
                        wo_t[kc][:, nn * 512:(nn + 1) * 512],
                        start=(kc == 0), stop=(kc == 1))
                nc.any.tensor_copy(stg[:, nn * 512:(nn + 1) * 512], ps[:])
            nc.sync.dma_start(out_d[sc * 128:(sc + 1) * 128, :], stg[:])

        groups = [(h, ic) for ic in reversed(range(S // IC)) for h in range(HPG)]
        pending = None
        for g in groups:
            at = attend(*g)
            if pending is not None:
                normalize(*pending)
                if pending[0] == HPG - 1:  # last head of its ic: mergedT
                    for sc in range(4 * pending[1], 4 * pending[1] + 4):
                        oproj(sc)          # cols for this ic fully written
            pending = (g[0], g[1], at)
        normalize(*pending)
        for sc in range(4 * pending[1], 4 * pending[1] + 4):
            oproj(sc)


_NC_CACHE = None


def _get_nc():
    global _NC_CACHE
    if _NC_CACHE is None:
        _NC_CACHE = _build_nc()
    return _NC_CACHE


def _causal_mask_tile():
    # BIGMASK[j, c] = 1.0 if j <= c - 384 else 0.0, shape [128, 896].
    # Diagonal block at delta = j_base - i_base uses cols [384-delta, 384-delta+512).
    j = np.arange(JC)[:, None]
    c = np.arange(896)[None, :]
    return (j <= c - 384).astype(np.float32)


def _prepare_in_maps(inputs):
    x = np.asarray(inputs["in_features"], dtype=np.float32)
    wqT = np.ascontiguousarray(np.asarray(inputs["q_proj_weight"], np.float32).T)
    wkT = np.ascontiguousarray(np.asarray(inputs["k_proj_weight"], np.float32).T)
    wvT = np.ascontiguousarray(np.asarray(inputs["v_proj_weight"], np.float32).T)
    woT = np.ascontiguousarray(np.asarray(inputs["o_proj_weight"], np.float32).T)
    xT = [np.ascontiguousarray(x[b].T) for b in range(B)]
    mask = _causal_mask_tile()

    in_maps = []
    for c in range(NCORES):
        b, g = divmod(c, HPG)
        ms = slice(g * M, (g + 1) * M)
        in_maps.append({
            "xT": xT[b],
            "wqT": np.ascontiguousarray(wqT[:, ms]),
            "wkT": np.ascontiguousarray(wkT[:, ms]),
            "wvT": np.ascontiguousarray(wvT[:, ms]),
            "woT": np.ascontiguousarray(woT[ms, :]),
            "mask": mask,
            "ones_a": np.ones((1, 64), np.float32),
            "ones_b": np.ones((JC, HPG), np.float32),
        })
    return in_maps


def kernel(q_proj_weight, k_proj_weight, v_proj_weight, o_proj_weight, in_features):
    in_dtype = np.asarray(in_features).dtype
    in_maps = _prepare_in_maps({
        "q_proj_weight": q_proj_weight,
        "k_proj_weight": k_proj_weight,
        "v_proj_weight": v_proj_weight,
        "o_proj_weight": o_proj_weight,
        "in_features": in_features,
    })
    nc = _get_nc()
    res = bass_utils.run_bass_kernel_spmd(nc, in_maps, core_ids=list(range(NCORES)))
    out = np.zeros((B, S, D), dtype=np.float32)
    for c in range(NCORES):
        out[c // HPG] += res.results[c]["out"]
    return out.astype(in_dtype)
